# revision 32
# baseline (speedup 1.0000x reference)
"""v3 of the 5x5 NN-loss kernel: three-engine rebalance (DVE/Act/Pool).

Same marshalled layout as v2 (see marshal_core). The 25 shifts are 13 sub
windows grouped into 8 units; units holding two homogeneous pair-groups
share one contiguous d tile so the adds and min tree run at doubled width.

v2 ran subs+adds+mins all on the DVE (79.4us of 2x-mode work) with abs on
Act (64us) hidden underneath -- DVE-walled at ~83us. v3 offloads most of
the channel-sum adds to the otherwise-idle Pool (gpsimd) engine, whose Q7
"Add" ucode runs tensor_tensor add/subtract at ~1.98 ns/elem. This drops
the DVE to subs + min tree + a rump of adds, landing all three engines at
~64-66us:

    DVE : 13 subs (76.8K elems) + min tree (24.5K) + ~15K adds
    Act : 76.8K abs (the hard wall: 0.833 ns/elem, no fast mode)
    Pool: ~36K adds at 1.98 ns/elem

Per-unit `adds_eng` config routes each unit's two merged adds to DVE or
Pool ('split' = u-add on Pool, s-add on DVE). The min tree stays on DVE
(the walrus ISA verifier rejects TensorTensor-min on Pool; Pool ucode
covers only Add/Multiply/Memset). abs_max and tensor_tensor_reduce are
also ISA-rejected, so no fused abs or fused final-reduce is available.

Emission is software-pipelined with lookahead 2 as in v2. g_odd is
derived on-chip from g_even (SBUF->SBUF DMA column shift + BIG pad memset
on Pool) and the head is split so the first j-half sub starts once pred
rows 0..6 and ge rows 0..6 land.
"""

import numpy as np

NCORES = 8
BIG = np.float16(20000.0)

GE_COLS = 3 * 8 * 260
P_COLS = 3 * 4 * 256

_cache = {}


def _build_nc(
    repeat=1,
    bufs=3,
    dbufs=5,
    lookahead=3,
    body_off=0,
    loads_off=0,
    go_onchip=1,
    head_split=1,
    adds_eng=None,
    steady_adds=None,
    subs_eng=None,
    steady_subs=None,
    act_tail=0,
    fold_delay=1,
):
    from contextlib import ExitStack

    import concourse.bacc as bacc
    import concourse.mybir as mybir
    import concourse.tile as tile
    from concourse.ap import AP

    f16 = mybir.dt.float16
    f32 = mybir.dt.float32
    Alu = mybir.AluOpType

    nc = bacc.Bacc("TRN2", target_bir_lowering=False)
    ge_d = nc.dram_tensor("g_even", [128, GE_COLS], f16, kind="ExternalInput")
    go_d = nc.dram_tensor("g_odd", [128, GE_COLS], f16, kind="ExternalInput")
    p_d = nc.dram_tensor("pred", [128, P_COLS], f16, kind="ExternalInput")
    out_d = nc.dram_tensor("out", [128, 1], f32, kind="ExternalOutput")

    # units: pair-groups sharing one contiguous d tile; merged downstream
    # ops run at doubled width with <=3 free dims (the group dim merges
    # uniformly with j -- unlike the sub windows, whose overlapping o dim
    # cannot merge, hence subs stay per-pair).
    # Unit order tunes the tail: Pool-fed (pd) units deliver their abs
    # early so Pool's adds fit its dependency window; the late positions
    # are split single-group units whose short (~3.4us) DVE chains keep
    # the post-last-abs drain small.
    UNITS = [
        [((-2,), (0, 2)), ((-1,), (0, 2))],
        [((0,), (0, 2)), ((1,), (0, 2))],
        [((-2, -1), (4,)), ((0, 1), (4,))],
        [((-2,), (1, 3)), ((-1,), (1, 3))],
        [((2,), (0, 2)), ((2,), (1, 3))],
        [((0,), (1, 3))],
        [((1,), (1, 3))],
        [((2,), (4,))],
    ]
    if adds_eng is None:
        # (u_eng, s_eng) per unit: every unit's u-add on Pool, s-add on
        # the DVE. Chain-free (Pool feeds the DVE, never blocks on its
        # own output) and measured fastest in steady state.
        adds_eng = ["pd"] * 8
    if steady_adds is None:
        # Middle repeats have no fill/drain window: every unit's u-add on
        # Pool (chain-free: Pool feeds the DVE's s-add, never blocks on
        # itself), s-adds and folds on the DVE.
        steady_adds = ["pd"] * 8
    if subs_eng is None:
        # U7's sub runs on Pool in its pre-first-abs idle hole (needs
        # only the ge loads); all other subs stay on the DVE.
        subs_eng = ["d", "d", "d", "d", "d", "d", "d", "p"]
    if steady_subs is None:
        # in steady state Pool's in-order queue must stay clear for the
        # u-adds; a sub parked there blocks them and stalls the DVE
        steady_subs = ["d"] * 8

    with ExitStack() as ctx:
        tc = ctx.enter_context(tile.TileContext(nc))
        pool = ctx.enter_context(tc.tile_pool(name="main", bufs=1))
        dpool = ctx.enter_context(tc.tile_pool(name="d", bufs=dbufs))
        spool = ctx.enter_context(tc.tile_pool(name="s", bufs=bufs))

        ge = pool.tile([128, GE_COLS], f16, tag="ge")
        go = pool.tile([128, GE_COLS], f16, tag="go")
        p = pool.tile([128, P_COLS], f16, tag="p")
        m = pool.tile([128, 1024], f16, tag="m")

        # pred on the Activation HWDGE queue, g tiles on SP: the two head
        # transfers run on different queues.
        band = 12 * 260
        if loads_off:
            nc.scalar.dma_start(out=p[:, 0:1], in_=p_d[:, 0:1])
            nc.sync.dma_start(out=ge[:, 0:1], in_=ge_d[:, 0:1])
            nc.sync.dma_start(out=go[:, 0:1], in_=go_d[:, 0:1])
        elif head_split:
            # finer head: the first head sub covers a single (j,c) row, so
            # only pred row 0 and ge row 0 gate the pipeline start. Both
            # go on the SP queue first (the Act queue's first issue waits
            # behind the abs-table load on the Act SEQ).
            nc.sync.dma_start(out=p[:, 0:256], in_=p_d[:, 0:256])
            nc.sync.dma_start(out=ge[:, 0:260], in_=ge_d[:, 0:260])
            nc.scalar.dma_start(out=p[:, 256:768], in_=p_d[:, 256:768])
            nc.sync.dma_start(out=ge[:, 260 : 3 * 260], in_=ge_d[:, 260 : 3 * 260])
            nc.scalar.dma_start(out=p[:, 768:1536], in_=p_d[:, 768:1536])
            nc.sync.dma_start(out=ge[:, 3 * 260 : 6 * 260], in_=ge_d[:, 3 * 260 : 6 * 260])
            nc.scalar.dma_start(out=p[:, 1536:P_COLS], in_=p_d[:, 1536:P_COLS])
            nc.sync.dma_start(out=ge[:, 6 * 260 : band], in_=ge_d[:, 6 * 260 : band])
            nc.sync.dma_start(out=ge[:, band:GE_COLS], in_=ge_d[:, band:GE_COLS])
            if not go_onchip:
                nc.sync.dma_start(out=go[:], in_=go_d[:])
        else:
            nc.scalar.dma_start(out=p[:], in_=p_d[:])
            nc.sync.dma_start(out=ge[:, 0:band], in_=ge_d[:, 0:band])
            nc.sync.dma_start(out=ge[:, band:GE_COLS], in_=ge_d[:, band:GE_COLS])
            if not go_onchip:
                nc.sync.dma_start(out=go[:], in_=go_d[:])

        if go_onchip and not loads_off:
            # go[r, 0:259] = ge[r, 1:260] via SBUF->SBUF DMA (no HBM
            # traffic, no compute-engine time); go[r, 259] is the constant
            # BIG pad column (memset on Pool, which is idle this early).
            src = AP(ge[:].tensor, 1, [[GE_COLS, 128], [260, 24], [1, 259]])
            dst = AP(go[:].tensor, 0, [[GE_COLS, 128], [260, 24], [1, 259]])
            nc.sync.dma_start(out=dst, in_=src)
            pad = AP(go[:].tensor, 259, [[GE_COLS, 128], [260, 24], [1, 1]])
            nc.gpsimd.memset(pad, float(BIG))

        ge_h = ge[:].tensor
        go_h = go[:].tensor
        p_h = p[:].tensor

        units = UNITS * repeat
        # first repeat is window-aware (Pool idles until the first abs
        # lands); middle/last repeats run the balanced steady split
        adds_cfg = adds_eng + steady_adds * (repeat - 1)
        subs_cfg = subs_eng + steady_subs * (repeat - 1)
        K = len(units)
        state = {}
        first = [True]

        def emit_sub(k, sub_only=False):
            sub_eng = nc.gpsimd if subs_cfg[k] == "p" else nc.vector
            unit = units[k]
            ntot = sum(len(o) * 12 * len(dj) * 256 for o, dj in unit)
            d = dpool.tile([128, ntot], f16, tag="d")
            off = 0
            for gi, (o_list, dj_list) in enumerate(unit):
                par = dj_list[0] % 2
                g_h = go_h if par else ge_h
                no, nk = len(o_list), len(dj_list)
                r0 = (o_list[0] + 2) * 3
                col = dj_list[0] - par
                n = no * 12 * nk * 256
                if head_split and k == 0 and gi == 0:
                    dview = d[:, off : off + n].rearrange(
                        "p (r k w) -> p r k w", r=12, k=nk, w=256
                    )
                    for rr, nr in ((0, 1), (1, 2), (3, 3), (6, 6)):
                        g_ap = AP(
                            g_h,
                            (r0 + rr) * 260 + col,
                            [[GE_COLS, 128], [260, nr], [2, nk], [1, 256]],
                        )
                        p_ap = AP(
                            p_h,
                            rr * 256,
                            [[P_COLS, 128], [256, nr], [0, nk], [1, 256]],
                        )
                        dslice = dview[:, rr : rr + nr]
                        sub_eng.tensor_tensor(dslice, g_ap, p_ap, Alu.subtract)
                        nc.scalar.activation(
                            dslice, dslice, mybir.ActivationFunctionType.Abs
                        )
                    off += n
                    continue
                if no == 1:
                    g_ap = AP(
                        g_h,
                        r0 * 260 + col,
                        [[GE_COLS, 128], [260, 12], [2, nk], [1, 256]],
                    )
                    p_ap = AP(
                        p_h, 0, [[P_COLS, 128], [256, 12], [0, nk], [1, 256]]
                    )
                    dview = d[:, off : off + n].rearrange(
                        "p (r k w) -> p r k w", r=12, k=nk, w=256
                    )
                else:
                    g_ap = AP(
                        g_h,
                        r0 * 260 + col,
                        [[GE_COLS, 128], [3 * 260, no], [260, 12], [1, 256]],
                    )
                    p_ap = AP(
                        p_h, 0, [[P_COLS, 128], [0, no], [256, 12], [1, 256]]
                    )
                    dview = d[:, off : off + n].rearrange(
                        "p (o r w) -> p o r w", o=no, r=12, w=256
                    )
                sub_eng.tensor_tensor(dview, g_ap, p_ap, Alu.subtract)
                if k == 0:
                    # head unit keeps per-member abs (finer early pipeline)
                    nc.scalar.activation(
                        d[:, off : off + n],
                        d[:, off : off + n],
                        mybir.ActivationFunctionType.Abs,
                    )
                off += n
            if k != 0 and not sub_only:
                # the merged adds read the whole unit tile, so one
                # unit-wide abs loses no dependency granularity
                nc.scalar.activation(
                    d[:, 0:ntot], d[:, 0:ntot], mybir.ActivationFunctionType.Abs
                )
            state[k] = d

        def emit_abs(k):
            d = state[k]
            nc.scalar.activation(
                d[:], d[:], mybir.ActivationFunctionType.Abs
            )

        def fold(cur, g, r):
            """min over the leading dim of cur viewed [p, g, r, 256]."""
            cv = cur[:].rearrange("p (g r w) -> p g r w", g=g, r=r, w=256)
            i0, i1 = cv[:, 0], cv[:, 1]
            if r == 4 and first[0]:
                ov = m[:].rearrange("p (r w) -> p r w", r=4, w=256)
                nc.vector.tensor_tensor(ov, i0, i1, Alu.min)
                first[0] = False
                return None
            nxt = spool.tile([128, r * 256], f16, tag=f"t{r}")
            ov = nxt[:].rearrange("p (r w) -> p r w", r=r, w=256)
            nc.vector.tensor_tensor(ov, i0, i1, Alu.min)
            return nxt

        sstate = {}

        def emit_adds(k):
            unit = units[k]
            d = state.pop(k)
            eng_cfg = adds_cfg[k]
            ng = len(unit)
            no, nk = len(unit[0][0]), len(unit[0][1])
            ns = ng * no * 4 * nk * 256
            s = spool.tile([128, ns], f16, tag="s")
            x = ng * no * 4
            if nk == 2:
                dv = d[:].rearrange(
                    "p (x c k w) -> p x c k w", x=x, c=3, k=2, w=256
                )
                sv = s[:].rearrange("p (x k w) -> p x k w", x=x, k=2, w=256)
            else:
                dv = d[:].rearrange("p (x c w) -> p x c w", x=x, c=3, w=256)
                sv = s[:].rearrange("p (x w) -> p x w", x=x, w=256)
            u_eng = nc.gpsimd if eng_cfg[0] == "p" else nc.vector
            s_eng = nc.gpsimd if eng_cfg[1] == "p" else nc.vector
            u_eng.tensor_tensor(sv, dv[:, :, 0], dv[:, :, 1], Alu.add)
            s_eng.tensor_tensor(sv, sv, dv[:, :, 2], Alu.add)
            sstate[k] = (s, sv, x, nk)

        def emit_folds(k, last):
            s, sv, x, nk = sstate.pop(k)
            # min tree down to [p, 1024] (DVE only: Pool has no min ucode)
            cur = s
            if nk == 2:
                # fold k first: [p, x, k, w] -> [p, x, w]
                i0, i1 = sv[:, :, 0], sv[:, :, 1]
                if x == 4 and first[0]:
                    ov = m[:].rearrange("p (r w) -> p r w", r=4, w=256)
                    nc.vector.tensor_tensor(ov, i0, i1, Alu.min)
                    first[0] = False
                    cur = None
                else:
                    cur = spool.tile([128, x * 256], f16, tag="tk")
                    ov = cur[:].rearrange("p (r w) -> p r w", r=x, w=256)
                    nc.vector.tensor_tensor(ov, i0, i1, Alu.min)
            rows = x
            while cur is not None and rows > 4:
                cur = fold(cur, 2, rows // 2)
                rows //= 2
            if cur is None:
                return
            src = cur[:]
            nc.vector.tensor_tensor(m[:], m[:], src, Alu.min)
            if last:
                r1 = pool.tile([128, 1], f32, tag="r1")
                if act_tail:
                    # final sum on Act via activation accum_out: frees the
                    # DVE tail and overlaps with its last min.
                    mc = spool.tile([128, 1024], f16, tag="mc")
                    nc.scalar.activation(
                        mc[:], m[:], mybir.ActivationFunctionType.Copy,
                        accum_out=r1[:],
                    )
                else:
                    nc.vector.tensor_reduce(
                        r1[:], m[:], mybir.AxisListType.X, Alu.add
                    )
                nc.sync.dma_start(out=out_d[:], in_=r1[:])

        if body_off:
            nc.vector.memset(m[:], 0)
            r1 = pool.tile([128, 1], f32, tag="r1")
            nc.vector.tensor_reduce(r1[:], m[:], mybir.AxisListType.X, Alu.add)
            nc.sync.dma_start(out=out_d[:], in_=r1[:])
        else:
            # Pool-assigned subs are emitted first: they depend only on
            # the loads, and Pool's in-order queue must not trap them
            # behind abs-gated adds.
            # only the first repeat's pool-subs pre-emit (they fill Pool's
            # pre-first-abs idle hole); later repeats' pool-subs flow in
            # their normal pipeline slots
            pre = [k for k in range(min(K, len(UNITS))) if subs_cfg[k] == "p"]
            for k in pre:
                emit_sub(k, sub_only=True)
            # staged emission: sub(k+LA) | adds(k) | folds(k-FD). The fold
            # delay keeps Pool-fed fold chains out of the DVE's in-order
            # queue until their s tiles are ready (no head-of-line block).
            for k in range(min(lookahead, K)):
                if k in pre:
                    emit_abs(k)
                else:
                    emit_sub(k)
            for k in range(K + fold_delay):
                ks = k + lookahead
                if ks < K:
                    if ks in pre:
                        emit_abs(ks)
                    else:
                        emit_sub(ks)
                if k < K:
                    emit_adds(k)
                kf = k - fold_delay
                if 0 <= kf < K:
                    emit_folds(kf, last=(kf == K - 1))

    if not nc.is_finalized():
        nc.finalize()
    return nc


def marshal_core(pred2, gt2):
    """pred2, gt2: [2, 3, 256, 256] f32 -> core input dict (fp16 layouts)."""
    gtp = np.full((2, 3, 260, 262), BIG, np.float16)
    gtp[:, :, 2:258, 2:258] = gt2.astype(np.float16)
    sw = np.lib.stride_tricks.sliding_window_view(gtp, 8, axis=2)  # [2,3,253,262,8]
    sel = sw[:, :, 0:253:4]  # rows 4g -> [2,3,64,262,8] = (b,c,g,w,jj)
    base = sel.transpose(0, 2, 4, 1, 3)  # (b,g,jj,c,w) = [2,64,8,3,262]
    ge = np.ascontiguousarray(base[..., 0:260]).reshape(128, GE_COLS)
    go = np.ascontiguousarray(base[..., 1:261]).reshape(128, GE_COLS)
    p16 = (
        pred2.astype(np.float16)
        .reshape(2, 3, 64, 4, 256)  # (b,c,g,j,w)
        .transpose(0, 2, 3, 1, 4)  # (b,g,j,c,w)
        .reshape(128, P_COLS)
    )
    return {
        "g_even": ge,
        "g_odd": go,
        "pred": np.ascontiguousarray(p16),
    }


def kernel(pred_target, gt_target):
    from concourse.bass_utils import run_bass_kernel_spmd

    pred_target = np.asarray(pred_target)
    gt_target = np.asarray(gt_target)

    if "nc" not in _cache:
        _cache["nc"] = _build_nc()
    nc = _cache["nc"]

    in_maps = [
        marshal_core(pred_target[2 * ci : 2 * ci + 2], gt_target[2 * ci : 2 * ci + 2])
        for ci in range(NCORES)
    ]
    try:
        res = run_bass_kernel_spmd(nc, in_maps, core_ids=list(range(NCORES)))
    except ModuleNotFoundError:
        import os

        os.environ["BASS_NEVER_TRACE"] = "1"
        res = run_bass_kernel_spmd(nc, in_maps, core_ids=list(range(NCORES)))
    total = 0.0
    for r in res.results:
        total += float(r["out"].astype(np.float64).sum())
    return np.float32(total)
